# revision 4
# baseline (speedup 1.0000x reference)
"""Self-contained Trainium2 Bass kernel for nn_Attention_395136991961 (final).

Dense MHA (B=8, N=1024, C=1024, H=16, D=64) with RoPE, full softmax,
output projection. Data-parallel over batch: one batch element per core.

v6: fine-grained software pipeline. Per-engine instruction issue is
in-order, so the per-pair schedule interleaves the NEXT pair's qkv
projection chains between the S and PV stages of the PREVIOUS pair --
PE filler runs while ACT chews through the exp backlog, and ACT's
copies slot into its exp gaps. All matmuls bf16 (FWL weight loads),
q/k projected directly transposed, RoPE via a constant rotation matmul
plus 3 DVE ops, softmax denominator as a ones-column in V, split
projection so 2/3 of proj streams before the last pair finishes.
"""

import sys

if "/opt/trn_rl_repo" not in sys.path:
    sys.path.insert(0, "/opt/trn_rl_repo")

import numpy as np

import concourse.tile as tile
import concourse.mybir as mybir
from concourse import bacc
from concourse.bass_utils import run_bass_kernel_spmd

F32 = mybir.dt.float32
BF16 = mybir.dt.bfloat16
AF = mybir.ActivationFunctionType
OP = mybir.AluOpType

N_CORES = 8
C = 1024
H = 16
D = 64
HD2 = D // 2
SCALE = float(D) ** -0.5

PROFILE = False
LAST_EXEC_NS = None
_CACHE = {}


def build(n_tok):
    ntile = n_tok // 128
    npair = H // 2
    nct = C // 128

    nc = bacc.Bacc("TRN2", target_bir_lowering=False, debug=False, num_devices=1)

    xT = nc.dram_tensor("xT", [128, nct * n_tok], BF16, kind="ExternalInput").ap()
    wqk = nc.dram_tensor("wqk", [128, 16 * nct * 128], BF16, kind="ExternalInput").ap()
    wv = nc.dram_tensor("wv", [128, 2 * nct * 512], BF16, kind="ExternalInput").ap()
    pwT = nc.dram_tensor("pwT", [128, nct * C], BF16, kind="ExternalInput").ap()
    pbias = nc.dram_tensor("pbias", [1, C], F32, kind="ExternalInput").ap()
    cosT2in = nc.dram_tensor("cosT2", [128, n_tok], BF16, kind="ExternalInput").ap()
    sinT2in = nc.dram_tensor("sinT2", [128, n_tok], BF16, kind="ExternalInput").ap()
    rotin = nc.dram_tensor("rotin", [128, 128], BF16, kind="ExternalInput").ap()
    y = nc.dram_tensor("y", [n_tok, C], BF16, kind="ExternalOutput").ap()

    xT_t = xT.rearrange("p (t n) -> p t n", t=nct)
    wqk_t = wqk.rearrange("p (jb t j) -> p jb t j", jb=16, t=nct)
    wv_t = wv.rearrange("p (ci t j) -> p ci t j", ci=2, t=nct)
    pwT_t = pwT.rearrange("p (t e) -> p t e", t=nct)

    with tile.TileContext(nc) as tc:
        with (
            tc.tile_pool(name="persist", bufs=1) as pp,
            tc.tile_pool(name="psA", bufs=2, space="PSUM") as psA,
            tc.tile_pool(name="psS", bufs=2, space="PSUM") as psS,
            tc.tile_pool(name="psO", bufs=2, space="PSUM") as psO,
        ):
            xT_sb = pp.tile([128, nct, n_tok], BF16, tag="xT")
            nc.sync.dma_start(xT_sb[:, :, 0:512], xT_t[:, :, 0:512])
            # PE warmup: dummy matmuls on a memset tile while input DMA
            # streams, so HAM is at full clock when real work starts
            warm_w = pp.tile([128, 512], BF16, tag="warmw")
            nc.vector.memset(warm_w[:], 0.0)
            rot_sb = pp.tile([128, 128], BF16, tag="rot")
            cosT2 = pp.tile([128, n_tok], BF16, tag="cos")
            sinT2 = pp.tile([128, n_tok], BF16, tag="sin")
            pw_sb = pp.tile([128, nct, C], BF16, tag="pw")
            pb_sb = pp.tile([1, C], F32, tag="pb")
            bias_b = pp.tile([128, C], F32, tag="biasb")

            # 4-slot rings (slot = pair % 4); k stored zero-padded to K=128
            # so S stationaries get FWL weight loads
            qT_sb = pp.tile([128, 4, n_tok], BF16, tag="qT")
            kTeZ = pp.tile([128, 4, n_tok], BF16, tag="kTeZ")
            kToZ = pp.tile([128, 4, n_tok], BF16, tag="kToZ")
            nc.vector.memset(kTeZ[64:128, :, :], 0.0)
            nc.vector.memset(kToZ[0:64, :, :], 0.0)
            v_sb = pp.tile([128, ntile, H, D + 1], BF16, tag="v")
            nc.vector.memset(v_sb[:, :, :, D : D + 1], 1.0)
            oT2 = pp.tile([128, nct, n_tok], BF16, tag="oT2")
            y1_sb = pp.tile([128, ntile, 2, 512], BF16, tag="y1")

            with (
                tc.tile_pool(name="wstream", bufs=3) as wsp,
                tc.tile_pool(name="wvpool", bufs=2) as wvp,
                tc.tile_pool(name="qraw", bufs=4) as qrp,
                tc.tile_pool(name="ropet", bufs=4) as rtp,
                tc.tile_pool(name="ptp", bufs=2) as ptp,
                tc.tile_pool(name="nrm", bufs=2) as nrm,
                tc.tile_pool(name="ypool", bufs=4) as yp,
            ):
                def fetch_w(jt, which):
                    jb = which * npair + jt
                    wtile = wsp.tile([128, nct, 128], BF16, tag="w")
                    nc.sync.dma_start(wtile[:], wqk_t[:, jb, :, :])
                    return wtile

                def qk_chain(wtile, qraw, mc2):
                    ms = mc2 * 512
                    pq = psA.tile([128, 512], F32, tag="pa")
                    for ct in range(nct):
                        nc.tensor.matmul(
                            pq[:],
                            wtile[:, ct, :],
                            xT_sb[:, ct, ms : ms + 512],
                            start=(ct == 0),
                            stop=(ct == nct - 1),
                        )
                    nc.vector.tensor_copy(qraw[:, ms : ms + 512], pq[:])

                def qk_rope(jt, which, qraw):
                    slot = jt % 4
                    for mc2 in range(2):
                        ms = mc2 * 512
                        prot = psA.tile([128, 512], F32, tag="pa")
                        nc.tensor.matmul(
                            prot[:], rot_sb[:], qraw[:, ms : ms + 512],
                            start=True, stop=True,
                        )
                        tt = rtp.tile([128, 512], BF16, tag="tt")
                        nc.vector.tensor_tensor(
                            out=tt[:], in0=qraw[:, ms : ms + 512],
                            in1=cosT2[:, ms : ms + 512], op=OP.mult,
                        )
                        uu = rtp.tile([128, 512], BF16, tag="uu")
                        nc.vector.tensor_tensor(
                            out=uu[:], in0=prot[:],
                            in1=sinT2[:, ms : ms + 512], op=OP.mult,
                        )
                        if which == 0:
                            nc.vector.tensor_tensor(
                                out=qT_sb[:, slot, ms : ms + 512],
                                in0=tt[:], in1=uu[:], op=OP.add,
                            )
                        else:
                            nc.vector.tensor_tensor(
                                out=kTeZ[0:64, slot, ms : ms + 512],
                                in0=tt[0:64, :], in1=uu[0:64, :], op=OP.add,
                            )
                            nc.vector.tensor_tensor(
                                out=kToZ[64:128, slot, ms : ms + 512],
                                in0=tt[64:128, :], in1=uu[64:128, :], op=OP.add,
                            )

                def v_tiles(wvt, ci, ts_):
                    for t in ts_:
                        pv = psA.tile([128, 512], F32, tag="pa")
                        for ct in range(nct):
                            nc.tensor.matmul(
                                pv[:],
                                xT_sb[:, ct, t * 128 : (t + 1) * 128],
                                wvt[:, ct, :],
                                start=(ct == 0),
                                stop=(ct == nct - 1),
                            )
                        nc.vector.tensor_copy(
                            v_sb[:, t, 8 * ci : 8 * ci + 8, 0:D],
                            pv[:].rearrange("p (h d) -> p h d", d=D),
                        )

                def attn_S_alloc():
                    pTe = ptp.tile([128, ntile, 512], BF16, tag="pTe")
                    pTo = ptp.tile([128, ntile, 512], BF16, tag="pTo")
                    return pTe, pTo

                def attn_S_tp(jt, mc, tp_, pT):
                    slot = jt % 4
                    ms = mc * 512
                    pTe, pTo = pT
                    pse = psS.tile([128, 2, 512], F32, tag="ps")
                    pso = psS.tile([128, 2, 512], F32, tag="ps")
                    for i in range(2):
                        t = 2 * tp_ + i
                        nc.tensor.matmul(
                            pse[:, i, :],
                            kTeZ[:, slot, t * 128 : (t + 1) * 128],
                            qT_sb[:, slot, ms : ms + 512],
                            start=True, stop=True,
                        )
                    for i in range(2):
                        t = 2 * tp_ + i
                        nc.tensor.matmul(
                            pso[:, i, :],
                            kToZ[:, slot, t * 128 : (t + 1) * 128],
                            qT_sb[:, slot, ms : ms + 512],
                            start=True, stop=True,
                        )
                    nc.scalar.activation(
                        pTe[:, 2 * tp_ : 2 * tp_ + 2, :].rearrange(
                            "p a m -> p (a m)"
                        ),
                        pse[:].rearrange("p a m -> p (a m)"),
                        AF.Exp, scale=SCALE,
                    )
                    nc.scalar.activation(
                        pTo[:, 2 * tp_ : 2 * tp_ + 2, :].rearrange(
                            "p a m -> p (a m)"
                        ),
                        pso[:].rearrange("p a m -> p (a m)"),
                        AF.Exp, scale=SCALE,
                    )

                def attn_PV(jt, mc, pTe, pTo):
                    ms = mc * 512
                    poE = psO.tile([65, 512], F32, tag="po")
                    poO = psO.tile([65, 512], F32, tag="po")
                    for t in range(ntile):
                        nc.tensor.matmul(
                            poE[:], v_sb[:, t, 2 * jt, :], pTe[:, t, :],
                            start=(t == 0), stop=(t == ntile - 1),
                        )
                    for t in range(ntile):
                        nc.tensor.matmul(
                            poO[:], v_sb[:, t, 2 * jt + 1, :], pTo[:, t, :],
                            start=(t == 0), stop=(t == ntile - 1),
                        )
                    for par, po in ((0, poE), (1, poO)):
                        # ACT moves the denominator row from psum partition 64
                        # to sbuf partition 0 (cross-partition copy verified
                        # on HW) -- no DMA hop
                        s0 = nrm.tile([1, 512], F32, tag="s0")
                        nc.scalar.copy(s0[0:1, :], po[64:65, :])
                        rs0 = nrm.tile([1, 512], F32, tag="rs0")
                        nc.vector.reciprocal_approx_fast(out=rs0[:], in_=s0[:])
                        rb_sb = nrm.tile([64, 512], F32, tag="rb")
                        nc.gpsimd.partition_broadcast(rb_sb[:], rs0[0:1, :])
                        nc.vector.tensor_tensor(
                            out=oT2[par * 64 : par * 64 + 64, jt, ms : ms + 512],
                            in0=po[0:64, :], in1=rb_sb[:], op=OP.mult,
                        )

                def proj_half1(t, ec):
                    py = psA.tile([128, 512], F32, tag="pa")
                    for ft in range(4):
                        nc.tensor.matmul(
                            py[:],
                            oT2[:, ft, t * 128 : (t + 1) * 128],
                            pw_sb[:, ft, ec * 512 : (ec + 1) * 512],
                            start=(ft == 0), stop=(ft == 3),
                        )
                    nc.vector.tensor_tensor(
                        out=y1_sb[:, t, ec, :], in0=py[:],
                        in1=bias_b[:, ec * 512 : (ec + 1) * 512], op=OP.add,
                    )

                def proj_half2(t, ec):
                    py = psA.tile([128, 512], F32, tag="pa")
                    for ft in range(4, nct):
                        nc.tensor.matmul(
                            py[:],
                            oT2[:, ft, t * 128 : (t + 1) * 128],
                            pw_sb[:, ft, ec * 512 : (ec + 1) * 512],
                            start=(ft == 4), stop=(ft == nct - 1),
                        )
                    ysb = yp.tile([128, 512], BF16, tag="y")
                    nc.vector.tensor_tensor(
                        out=ysb[:], in0=py[:], in1=y1_sb[:, t, ec, :], op=OP.add,
                    )
                    nc.sync.dma_start(
                        y[t * 128 : (t + 1) * 128, ec * 512 : (ec + 1) * 512],
                        ysb[:],
                    )

                # ---------------- schedule ----------------
                wq0 = fetch_w(0, 0)
                wk0 = fetch_w(0, 1)
                nc.sync.dma_start(rot_sb[:], rotin[:])
                nc.sync.dma_start(xT_sb[:, :, 512:1024], xT_t[:, :, 512:1024])

                # warmup matmuls (results unused) bridge the input-DMA wait
                # so HAM is at full clock when real work starts
                for _ in range(5):
                    pwm = psS.tile([128, 2, 512], F32, tag="ps")
                    for i in range(8):
                        nc.tensor.matmul(
                            pwm[:, i % 2, :], warm_w[:, 0:128], warm_w[:],
                            start=True, stop=True,
                        )

                # cycle 0: project pair 0 (no attn yet)
                qA = qrp.tile([128, n_tok], BF16, tag="qraw")
                qB = qrp.tile([128, n_tok], BF16, tag="qraw")
                qk_chain(wq0, qA, 0)
                qk_chain(wk0, qB, 0)
                nc.sync.dma_start(cosT2[:], cosT2in[:])
                nc.sync.dma_start(sinT2[:], sinT2in[:])
                wvt0 = wvp.tile([128, nct, 512], BF16, tag="wv")
                nc.sync.dma_start(wvt0[:], wv_t[:, 0, :, :])
                wvt1 = wvp.tile([128, nct, 512], BF16, tag="wv")
                nc.sync.dma_start(wvt1[:], wv_t[:, 1, :, :])
                qk_chain(wq0, qA, 1)
                qk_chain(wk0, qB, 1)
                qk_rope(0, 0, qA)
                qk_rope(0, 1, qB)

                proj1_chunks = [(t, ec) for t in range(ntile) for ec in range(2)]

                def filler(cyc, slot3, wvt1):
                    # one small PE job (~1-2us) to pad an S tp-gap
                    if cyc == 2:
                        v_tiles(wvt1, 1, [slot3])
                    elif cyc == 3:
                        v_tiles(wvt1, 1, [3 + slot3])
                    elif cyc == 4:
                        v_tiles(wvt1, 1, [6 + slot3] if slot3 < 2 else [])
                        if slot3 == 2 and proj1_chunks:
                            t, ec = proj1_chunks.pop(0)
                            proj_half1(t, ec)
                    elif cyc >= 5:
                        if proj1_chunks:
                            t, ec = proj1_chunks.pop(0)
                            proj_half1(t, ec)

                for cyc in range(1, npair):
                    prev = cyc - 1
                    pT0 = attn_S_alloc()
                    pT1 = attn_S_alloc()
                    wq = fetch_w(cyc, 0)
                    wk = fetch_w(cyc, 1)
                    qA = qrp.tile([128, n_tok], BF16, tag="qraw")
                    qB = qrp.tile([128, n_tok], BF16, tag="qraw")
                    attn_S_tp(prev, 0, 0, pT0)
                    qk_chain(wq, qA, 0)
                    attn_S_tp(prev, 0, 1, pT0)
                    qk_chain(wq, qA, 1)
                    attn_S_tp(prev, 0, 2, pT0)
                    qk_chain(wk, qB, 0)
                    attn_S_tp(prev, 0, 3, pT0)
                    if cyc == 1:
                        v_tiles(wvt0, 0, range(ntile))
                        qk_chain(wk, qB, 1)
                        qk_rope(cyc, 0, qA)
                        attn_PV(prev, 0, *pT0)
                    else:
                        qk_chain(wk, qB, 1)
                        qk_rope(cyc, 0, qA)
                        attn_PV(prev, 0, *pT0)
                    if cyc == 2:
                        nc.sync.dma_start(pw_sb[:], pwT_t)
                        nc.sync.dma_start(pb_sb[:], pbias[:])
                        nc.gpsimd.partition_broadcast(bias_b[:], pb_sb[0:1, :])
                    attn_S_tp(prev, 1, 0, pT1)
                    qk_rope(cyc, 1, qB)
                    attn_S_tp(prev, 1, 1, pT1)
                    filler(cyc, 0, wvt1)
                    attn_S_tp(prev, 1, 2, pT1)
                    filler(cyc, 1, wvt1)
                    attn_S_tp(prev, 1, 3, pT1)
                    filler(cyc, 2, wvt1)
                    attn_PV(prev, 1, *pT1)

                # tail: attn of pair 7 with proj1 leftovers as PE filler
                last = npair - 1
                pT0 = attn_S_alloc()
                pT1 = attn_S_alloc()
                attn_S_tp(last, 0, 0, pT0)
                attn_S_tp(last, 0, 1, pT0)
                t, ec = proj1_chunks.pop(0)
                proj_half1(t, ec)
                attn_S_tp(last, 0, 2, pT0)
                t, ec = proj1_chunks.pop(0)
                proj_half1(t, ec)
                attn_S_tp(last, 0, 3, pT0)
                t, ec = proj1_chunks.pop(0)
                proj_half1(t, ec)
                attn_PV(last, 0, *pT0)
                attn_S_tp(last, 1, 0, pT1)
                attn_S_tp(last, 1, 1, pT1)
                t, ec = proj1_chunks.pop(0)
                proj_half1(t, ec)
                attn_S_tp(last, 1, 2, pT1)
                t, ec = proj1_chunks.pop(0)
                proj_half1(t, ec)
                attn_S_tp(last, 1, 3, pT1)
                for t, ec in proj1_chunks:
                    proj_half1(t, ec)
                attn_PV(last, 1, *pT1)
                # tokens 0:512 of proj2 depend only on pair-7 mc0 norms --
                # their matmuls fill the norm(7,mc1) latency
                for t in range(ntile // 2):
                    for ec in range(2):
                        proj_half2(t, ec)
                for t in range(ntile // 2, ntile):
                    for ec in range(2):
                        proj_half2(t, ec)

    nc.compile()
    return nc


def _host_inputs(x, rope_freqs, qkv_w, proj_w, proj_b):
    import ml_dtypes

    x = np.asarray(x, dtype=np.float32)
    rope_freqs = np.asarray(rope_freqs, dtype=np.float32)
    qkv_w = np.asarray(qkv_w, dtype=np.float32)
    proj_w = np.asarray(proj_w, dtype=np.float32)
    proj_b = np.asarray(proj_b, dtype=np.float32)

    B, n_tok, _ = x.shape
    bf = ml_dtypes.bfloat16
    wqk_h = np.ascontiguousarray(
        qkv_w[: 2 * C].reshape(16, 128, 8, 128).transpose(3, 0, 2, 1).reshape(128, -1)
    ).astype(bf)
    wv_h = np.ascontiguousarray(
        qkv_w[2 * C :].reshape(2, 512, 8, 128).transpose(3, 0, 2, 1).reshape(128, -1)
    ).astype(bf)
    pwTh = np.ascontiguousarray(
        proj_w.T.reshape(8, 128, C).transpose(1, 0, 2).reshape(128, -1)
    ).astype(bf)
    freqs = rope_freqs[0, :, 0, :]
    cos = np.cos(freqs)
    sin = np.sin(freqs)
    cosT2 = np.ascontiguousarray(np.tile(cos.T, (2, 1))).astype(bf)
    sinT2 = np.ascontiguousarray(np.tile(sin.T, (2, 1))).astype(bf)
    r = np.zeros((64, 64), dtype=np.float32)
    for j in range(HD2):
        r[j + HD2, j] = -1.0
    for j in range(HD2, D):
        r[j - HD2, j] = 1.0
    rot128 = np.zeros((128, 128), dtype=np.float32)
    rot128[0:64, 0:64] = r
    rot128[64:128, 64:128] = r
    rot128 = rot128.astype(bf)
    pbh = np.ascontiguousarray(proj_b.reshape(1, C))

    in_maps = []
    for b in range(B):
        xTh = np.ascontiguousarray(
            x[b].T.reshape(8, 128, n_tok).transpose(1, 0, 2).reshape(128, -1)
        ).astype(bf)
        in_maps.append(
            {
                "xT": xTh,
                "wqk": wqk_h,
                "wv": wv_h,
                "pwT": pwTh,
                "pbias": pbh,
                "cosT2": cosT2,
                "sinT2": sinT2,
                "rotin": rot128,
            }
        )
    return in_maps, n_tok


def kernel(x, rope_freqs, qkv_w, proj_w, proj_b):
    global LAST_EXEC_NS
    in_maps, n_tok = _host_inputs(x, rope_freqs, qkv_w, proj_w, proj_b)
    key = ("nc", n_tok)
    if key not in _CACHE:
        _CACHE[key] = build(n_tok)
    nc = _CACHE[key]

    trace = False
    if PROFILE:
        try:
            import profshim

            profshim.install()
            trace = True
        except Exception:
            trace = False

    res = run_bass_kernel_spmd(nc, in_maps, list(range(len(in_maps))), trace=trace)
    LAST_EXEC_NS = res.exec_time_ns
    out = np.stack(
        [res.results[b]["y"].astype(np.float32) for b in range(len(in_maps))], axis=0
    )
    return out
